# revision 4
# baseline (speedup 1.0000x reference)
"""MipNeRF renderer kernel for 8 Trainium2 NeuronCores.

Strategy: embarrassingly parallel over rays (data parallel, leading axis
sharded 8 ways via jax.pmap on the neuron PJRT backend).

trn2-specific rewrites vs the reference:
- jnp.sort (unsupported on trn2) -> full-width lax.top_k (supported),
  ascending sort = -top_k(-x).
- take_along_axis gathers (IndirectLoad overflows a 16-bit ISA field at
  this size) -> masked max/min reductions, exact because cdf and t_vals
  are sorted ascending: the gathered prefix-max / suffix-min equals the
  indexed element.
"""

import numpy as np
import jax
import jax.numpy as jnp
from jax import lax

N_RAYS = 65536
NUM_SAMPLES = 64      # coarse samples per ray (sorted t_vals)
NUM_IMPORTANCE = 128  # importance samples per ray
S = NUM_SAMPLES + NUM_IMPORTANCE  # 192
N_CORES = 8
BIG = 1e30


def _render_shard(t_vals, weights, u, densities, colors):
    # --- hierarchical (inverse-CDF) sampling ---
    w = weights[..., 1:-1] + 1e-5                       # [n, 62]
    pdf = w / jnp.sum(w, axis=-1, keepdims=True)
    cdf = jnp.cumsum(pdf, axis=-1)
    cdf = jnp.concatenate([jnp.zeros_like(cdf[..., :1]), cdf], axis=-1)  # [n, 63]
    t63 = t_vals[..., :63]

    # searchsorted(side='right') gives below = inds-1, above = inds (clipped
    # to 62).  Instead of gathering cdf/t_vals at those indices, exploit
    # sortedness:
    #   cdf[below]  = max{cdf_k : cdf_k <= u}            (k=0 always in set)
    #   t63[below]  = max{t63_k : cdf_k <= u}            (t63 ascending, >0)
    #   cdf[above]  = min(min{cdf_k : cdf_k > u}, cdf[62])
    #   t63[above]  = min(min{t63_k : cdf_k > u}, t63[62])
    cdf_b = cdf[:, None, :]                              # [n, 1, 63]
    t_b = t63[:, None, :]
    u_b = u[:, :, None]                                  # [n, 128, 1]
    mask = cdf_b <= u_b                                  # [n, 128, 63]
    cdf_g0 = jnp.max(jnp.where(mask, cdf_b, 0.0), axis=-1)
    bins_g0 = jnp.max(jnp.where(mask, t_b, 0.0), axis=-1)
    cdf_g1 = jnp.minimum(
        jnp.min(jnp.where(mask, BIG, cdf_b), axis=-1), cdf[:, -1:]
    )
    bins_g1 = jnp.minimum(
        jnp.min(jnp.where(mask, BIG, t_b), axis=-1), t63[:, -1:]
    )

    denom = cdf_g1 - cdf_g0
    denom = jnp.where(denom < 1e-5, jnp.ones_like(denom), denom)
    t = (u - cdf_g0) / denom
    samples = bins_g0 + t * (bins_g1 - bins_g0)          # [n, 128]
    samples = lax.optimization_barrier(samples)

    # --- merge + sort via full-width top_k (ascending = -top_k(-x)) ---
    merged = jnp.concatenate([t_vals, samples], axis=-1)  # [n, 192]
    merged = lax.optimization_barrier(merged)
    neg_sorted, _ = lax.top_k(-merged, S)
    t_all = -neg_sorted                                   # [n, 192] ascending
    t_all = lax.optimization_barrier(t_all)

    # --- volumetric rendering ---
    dists = jnp.diff(t_all, axis=-1)
    dists = jnp.concatenate([dists, jnp.full_like(dists[..., :1], 1e10)], axis=-1)
    sigma = jax.nn.relu(densities[..., 0])               # [n, 192]
    # clip the exponent: exp(-x) underflows to 0 for x>88 anyway, but the
    # hardware exp LUT returns garbage for extreme inputs (x ~ 1e10 from the
    # final 1e10 dist).  exp(-80) ~ 1.8e-35 is indistinguishable from 0.
    alpha = 1.0 - jnp.exp(-jnp.minimum(sigma * dists, 80.0))
    trans = jnp.cumprod(1.0 - alpha + 1e-10, axis=-1)
    trans = jnp.concatenate([jnp.ones_like(trans[..., :1]), trans[..., :-1]], axis=-1)
    wgt = alpha * trans                                   # [n, 192]
    rgb = jnp.sum(wgt[..., None] * colors, axis=-2)       # [n, 3]
    depth = jnp.sum(wgt * t_all, axis=-1)                 # [n]
    acc_alpha = jnp.sum(wgt, axis=-1)                     # [n]
    return rgb, depth, acc_alpha, wgt


_pmapped = None


def _get_pmapped():
    global _pmapped
    if _pmapped is None:
        _pmapped = jax.pmap(_render_shard, devices=jax.devices()[:N_CORES])
    return _pmapped


def kernel(t_vals, weights, u, densities, colors):
    def shard(x):
        return np.ascontiguousarray(
            x.reshape((N_CORES, x.shape[0] // N_CORES) + x.shape[1:])
        )

    fn = _get_pmapped()
    rgb, depth, acc, wgt = fn(
        shard(t_vals), shard(weights), shard(u), shard(densities), shard(colors)
    )
    rgb = np.asarray(rgb).reshape(N_RAYS, 3)
    depth = np.asarray(depth).reshape(N_RAYS)
    acc = np.asarray(acc).reshape(N_RAYS)
    wgt = np.asarray(wgt).reshape(N_RAYS, S)
    return rgb, depth, acc, wgt


# revision 6
# speedup vs baseline: 345.7252x; 345.7252x over previous
"""MipNeRF renderer kernel for 8 Trainium2 NeuronCores.

Strategy: embarrassingly parallel over rays (data parallel, leading axis
sharded 8 ways via jax.pmap on the neuron PJRT backend).

trn2-specific rewrites vs the reference:
- jnp.sort (unsupported on trn2) -> full-width lax.top_k (supported),
  ascending sort = -top_k(-x).
- take_along_axis gathers (IndirectLoad overflows a 16-bit ISA field at
  this size) -> masked max/min reductions, exact because cdf and t_vals
  are sorted ascending: the gathered prefix-max / suffix-min equals the
  indexed element.
"""

import numpy as np
import jax
import jax.numpy as jnp
from jax import lax

N_RAYS = 65536
NUM_SAMPLES = 64      # coarse samples per ray (sorted t_vals)
NUM_IMPORTANCE = 128  # importance samples per ray
S = NUM_SAMPLES + NUM_IMPORTANCE  # 192
N_CORES = 8
BIG = 1e30


def _render_shard(t_vals, weights, u, densities, colors):
    # --- hierarchical (inverse-CDF) sampling ---
    w = weights[..., 1:-1] + 1e-5                       # [n, 62]
    pdf = w / jnp.sum(w, axis=-1, keepdims=True)
    # cdf_full[k] = sum_{i<k} pdf_i, k=0..62 — via one PE matmul instead of a
    # scan lowering (cumsum scans are slow on trn2)
    tri_exc = jnp.asarray(
        np.triu(np.ones((62, 63), dtype=np.float32), k=1)
    )  # [62,63], T[i,k]=1 iff i<k
    cdf = pdf @ tri_exc                                  # [n, 63], cdf[...,0]=0
    t63 = t_vals[..., :63]

    # searchsorted(side='right') gives below = inds-1, above = inds (clipped
    # to 62).  Instead of gathering cdf/t_vals at those indices, exploit
    # sortedness:
    #   cdf[below]  = max{cdf_k : cdf_k <= u}            (k=0 always in set)
    #   t63[below]  = max{t63_k : cdf_k <= u}            (t63 ascending, >0)
    #   cdf[above]  = min(min{cdf_k : cdf_k > u}, cdf[62])
    #   t63[above]  = min(min{t63_k : cdf_k > u}, t63[62])
    cdf_b = cdf[:, None, :]                              # [n, 1, 63]
    t_b = t63[:, None, :]
    u_b = u[:, :, None]                                  # [n, 128, 1]
    mask = cdf_b <= u_b                                  # [n, 128, 63]
    cdf_g0 = jnp.max(jnp.where(mask, cdf_b, 0.0), axis=-1)
    bins_g0 = jnp.max(jnp.where(mask, t_b, 0.0), axis=-1)
    cdf_g1 = jnp.minimum(
        jnp.min(jnp.where(mask, BIG, cdf_b), axis=-1), cdf[:, -1:]
    )
    bins_g1 = jnp.minimum(
        jnp.min(jnp.where(mask, BIG, t_b), axis=-1), t63[:, -1:]
    )

    denom = cdf_g1 - cdf_g0
    denom = jnp.where(denom < 1e-5, jnp.ones_like(denom), denom)
    t = (u - cdf_g0) / denom
    samples = bins_g0 + t * (bins_g1 - bins_g0)          # [n, 128]
    samples = lax.optimization_barrier(samples)

    # --- merge + sort via full-width top_k (ascending = -top_k(-x)) ---
    merged = jnp.concatenate([t_vals, samples], axis=-1)  # [n, 192]
    merged = lax.optimization_barrier(merged)
    neg_sorted, _ = lax.top_k(-merged, S)
    t_all = -neg_sorted                                   # [n, 192] ascending
    t_all = lax.optimization_barrier(t_all)

    # --- volumetric rendering ---
    dists = jnp.diff(t_all, axis=-1)
    dists = jnp.concatenate([dists, jnp.full_like(dists[..., :1], 1e10)], axis=-1)
    sigma = jax.nn.relu(densities[..., 0])               # [n, 192]
    # clip the exponent: exp(-x) underflows to 0 for x>88 anyway, but the
    # hardware exp LUT returns garbage for extreme inputs (x ~ 1e10 from the
    # final 1e10 dist).  exp(-80) ~ 1.8e-35 is indistinguishable from 0.
    sd = jnp.minimum(sigma * dists, 80.0)
    alpha = 1.0 - jnp.exp(-sd)
    # exclusive cumprod(1-alpha+1e-10) = exp(exclusive-cumsum(log(...)));
    # the cumsum is a PE matmul against a strictly-upper-triangular ones
    # matrix (exp/log roundtrip error ~1e-6 rel, well inside tolerance).
    logx = jnp.log(jnp.exp(-sd) + 1e-10)                 # [n, 192]
    tri192 = jnp.asarray(
        np.triu(np.ones((S, S), dtype=np.float32), k=1)
    )
    trans = jnp.exp(logx @ tri192)                       # [n, 192], trans[...,0]=1
    wgt = alpha * trans                                   # [n, 192]
    rgb = jnp.sum(wgt[..., None] * colors, axis=-2)       # [n, 3]
    depth = jnp.sum(wgt * t_all, axis=-1)                 # [n]
    acc_alpha = jnp.sum(wgt, axis=-1)                     # [n]
    return rgb, depth, acc_alpha, wgt


_pmapped = None


def _get_pmapped():
    global _pmapped
    if _pmapped is None:
        _pmapped = jax.pmap(_render_shard, devices=jax.devices()[:N_CORES])
    return _pmapped


def kernel(t_vals, weights, u, densities, colors):
    def shard(x):
        return np.ascontiguousarray(
            x.reshape((N_CORES, x.shape[0] // N_CORES) + x.shape[1:])
        )

    fn = _get_pmapped()
    rgb, depth, acc, wgt = fn(
        shard(t_vals), shard(weights), shard(u), shard(densities), shard(colors)
    )
    rgb = np.asarray(rgb).reshape(N_RAYS, 3)
    depth = np.asarray(depth).reshape(N_RAYS)
    acc = np.asarray(acc).reshape(N_RAYS)
    wgt = np.asarray(wgt).reshape(N_RAYS, S)
    return rgb, depth, acc, wgt


# revision 7
# speedup vs baseline: 426.3972x; 1.2333x over previous
"""MipNeRF renderer kernel for 8 Trainium2 NeuronCores.

Strategy: embarrassingly parallel over rays (data parallel, leading axis
sharded 8 ways via jax.pmap on the neuron PJRT backend).

trn2-specific rewrites vs the reference:
- jnp.sort (unsupported on trn2) -> full-width lax.top_k (supported),
  ascending sort = -top_k(-x).
- take_along_axis gathers (IndirectLoad overflows a 16-bit ISA field at
  this size) -> masked max/min reductions, exact because cdf and t_vals
  are sorted ascending: the gathered prefix-max / suffix-min equals the
  indexed element.
"""

import numpy as np
import jax
import jax.numpy as jnp
from jax import lax

N_RAYS = 65536
NUM_SAMPLES = 64      # coarse samples per ray (sorted t_vals)
NUM_IMPORTANCE = 128  # importance samples per ray
S = NUM_SAMPLES + NUM_IMPORTANCE  # 192
N_CORES = 8
BIG = 1e30


def _render_shard(t_vals, weights, u, densities, colors):
    # --- hierarchical (inverse-CDF) sampling ---
    w = weights[..., 1:-1] + 1e-5                       # [n, 62]
    pdf = w / jnp.sum(w, axis=-1, keepdims=True)
    # cdf_full[k] = sum_{i<k} pdf_i, k=0..62 — via one PE matmul instead of a
    # scan lowering (cumsum scans are slow on trn2)
    tri_exc = jnp.asarray(
        np.triu(np.ones((62, 63), dtype=np.float32), k=1)
    )  # [62,63], T[i,k]=1 iff i<k
    cdf = pdf @ tri_exc                                  # [n, 63], cdf[...,0]=0
    t63 = t_vals[..., :63]

    # searchsorted(side='right') gives below = inds-1, above = inds (clipped
    # to 62).  Instead of gathering cdf/t_vals at those indices, exploit
    # sortedness: with mask_k = [cdf_k <= u] (a prefix 0..below),
    #   cdf[below]  = max(mask * cdf)          (cdf >= 0, k=0 always in set)
    #   t63[below]  = max(mask * t63)          (t63 ascending, > 0)
    #   cdf[above]  = max(mask * cdf_next)     cdf_next[k] = cdf[min(k+1, 62)]
    #   t63[above]  = max(mask * t63_next)     likewise shifted
    # All four share ONE mask; the ray axis is chunked so each chunk's
    # [chunk, 128, 63] mask stays on-chip instead of round-tripping HBM.
    cdf_next = jnp.concatenate([cdf[..., 1:], cdf[..., -1:]], axis=-1)
    t63_next = jnp.concatenate([t63[..., 1:], t63[..., -1:]], axis=-1)

    n = u.shape[0]
    CH = 256
    g0c, b0c, g1c, b1c = [], [], [], []
    for i in range(0, n, CH):
        sl = slice(i, i + CH)
        mask = cdf[sl, None, :] <= u[sl, :, None]        # [CH, 128, 63]
        g0c.append(jnp.max(jnp.where(mask, cdf[sl, None, :], 0.0), axis=-1))
        b0c.append(jnp.max(jnp.where(mask, t63[sl, None, :], 0.0), axis=-1))
        g1c.append(jnp.max(jnp.where(mask, cdf_next[sl, None, :], 0.0), axis=-1))
        b1c.append(jnp.max(jnp.where(mask, t63_next[sl, None, :], 0.0), axis=-1))
    cdf_g0 = jnp.concatenate(g0c, axis=0)
    bins_g0 = jnp.concatenate(b0c, axis=0)
    cdf_g1 = jnp.concatenate(g1c, axis=0)
    bins_g1 = jnp.concatenate(b1c, axis=0)

    denom = cdf_g1 - cdf_g0
    denom = jnp.where(denom < 1e-5, jnp.ones_like(denom), denom)
    t = (u - cdf_g0) / denom
    samples = bins_g0 + t * (bins_g1 - bins_g0)          # [n, 128]
    samples = lax.optimization_barrier(samples)

    # --- merge + sort via full-width top_k (ascending = -top_k(-x)) ---
    merged = jnp.concatenate([t_vals, samples], axis=-1)  # [n, 192]
    merged = lax.optimization_barrier(merged)
    neg_sorted, _ = lax.top_k(-merged, S)
    t_all = -neg_sorted                                   # [n, 192] ascending
    t_all = lax.optimization_barrier(t_all)

    # --- volumetric rendering ---
    dists = jnp.diff(t_all, axis=-1)
    dists = jnp.concatenate([dists, jnp.full_like(dists[..., :1], 1e10)], axis=-1)
    sigma = jax.nn.relu(densities[..., 0])               # [n, 192]
    # clip the exponent: exp(-x) underflows to 0 for x>88 anyway, but the
    # hardware exp LUT returns garbage for extreme inputs (x ~ 1e10 from the
    # final 1e10 dist).  exp(-80) ~ 1.8e-35 is indistinguishable from 0.
    sd = jnp.minimum(sigma * dists, 80.0)
    alpha = 1.0 - jnp.exp(-sd)
    # exclusive cumprod(1-alpha+1e-10) = exp(exclusive-cumsum(log(...)));
    # the cumsum is a PE matmul against a strictly-upper-triangular ones
    # matrix (exp/log roundtrip error ~1e-6 rel, well inside tolerance).
    logx = jnp.log(jnp.exp(-sd) + 1e-10)                 # [n, 192]
    tri192 = jnp.asarray(
        np.triu(np.ones((S, S), dtype=np.float32), k=1)
    )
    trans = jnp.exp(logx @ tri192)                       # [n, 192], trans[...,0]=1
    wgt = alpha * trans                                   # [n, 192]
    rgb = jnp.sum(wgt[..., None] * colors, axis=-2)       # [n, 3]
    depth = jnp.sum(wgt * t_all, axis=-1)                 # [n]
    acc_alpha = jnp.sum(wgt, axis=-1)                     # [n]
    return rgb, depth, acc_alpha, wgt


_pmapped = None


def _get_pmapped():
    global _pmapped
    if _pmapped is None:
        _pmapped = jax.pmap(_render_shard, devices=jax.devices()[:N_CORES])
    return _pmapped


def kernel(t_vals, weights, u, densities, colors):
    def shard(x):
        return np.ascontiguousarray(
            x.reshape((N_CORES, x.shape[0] // N_CORES) + x.shape[1:])
        )

    fn = _get_pmapped()
    rgb, depth, acc, wgt = fn(
        shard(t_vals), shard(weights), shard(u), shard(densities), shard(colors)
    )
    rgb = np.asarray(rgb).reshape(N_RAYS, 3)
    depth = np.asarray(depth).reshape(N_RAYS)
    acc = np.asarray(acc).reshape(N_RAYS)
    wgt = np.asarray(wgt).reshape(N_RAYS, S)
    return rgb, depth, acc, wgt
